# revision 1
# baseline (speedup 1.0000x reference)
"""3-layer GCN (PyG GCNConv-style) on 8 Trainium2 NeuronCores via Bass/Tile.

reference:  h1 = sigmoid(gcn(x,  W1,b1));  h2 = sigmoid(gcn(h1, W2,b2))
            h3 = gcn(h2, W3,b3);           out = sigmoid(h3 @ Wlin + blin)
with gcn(x,W,b) = D^-1/2 (A+I) D^-1/2 (x W) + b.

The symmetric normalization factorizes per edge (norm_e = dinv[src]*dinv[dst]):
    table = dinv * (x W)                        (per-node row scale)
    sum_d = sum_{e: dst(e)=d} table[src(e)]     (self loop included as edge)
    out_d = dinv[d]*sum_d + b                   (then sigmoid)

Sharding: nodes block-partitioned across 8 cores. Each core computes xW for
its own nodes, prescales by dinv, AllGathers the per-layer node table
(3.2MB/core), then gathers + reduces its own in-edges.

Gather: `dma_gather` (int16 indices, 256B-encoded row stride). The f32 table
is viewed as PAIR-rows (64 f32 = 256B stride) so an int16 index reaches 32768
pairs = 65536 nodes; the 200704-row table is covered by 4 windows x 2
parities = 8 edge categories. in_ap base selects (window, parity);
elem_size=32 moves exactly one 128B node row per index.

Reduce: per category, edges sorted by dst, split into cells of 64 consecutive
dst nodes. Each cell gets a fixed (max over cores — all 8 cores must run the
same program) number of 128-edge columns; pad slots gather an all-zero row.
A column's messages [128e, 32f] are scatter-reduced by a PE matmul against a
selection matrix S[e,d] = (dstl[e] == d) built with one DVE is_equal against
a constant iota row. PSUM accumulates per (category, 128-dst block); partials
are copied/added into a per-layer SBUF accumulator [128, n_chunks*32].
"""

import os
import textwrap
import inspect
import numpy as np

import concourse.bass as bass
import concourse.bacc as bacc
import concourse.mybir as mybir
import concourse.tile as tile
from concourse import bass_utils
from concourse.masks import make_identity

F32 = mybir.dt.float32
I16 = mybir.dt.int16

C = 8          # cores
P = 128        # partitions
F_IN = 128
H = 32
CELL = 64      # dst nodes per selection cell (S width)
WIN_PAIRS = 32768   # pair-rows reachable by one int16 index
TCOLS = 64     # columns (of 128 edges) per dma_gather call
# 64 cols = 8192 indices = 513 SWDGE descriptors; the descriptor ring
# holds ~1024 and a single packet only 64 (so single_packet=False).


def _patch_dma_gather():
    """bass.dma_gather asserts elem_size_bytes % 256 == 0 (a transpose-path
    restriction applied unconditionally). The ucode handles 64B/128B elements
    with a 256B-encoded stride (verified on device); relax to %64."""
    if getattr(bass.BassGpSimd.dma_gather, "_relaxed", False):
        return
    src = textwrap.dedent(inspect.getsource(bass.BassGpSimd.dma_gather))
    assert "elem_size_bytes % 256 == 0" in src
    src = src.replace("elem_size_bytes % 256 == 0", "elem_size_bytes % 64 == 0")
    ns = {}
    exec(compile(src, "<dma_gather_patched>", "exec"), vars(bass).copy(), ns)
    fn = ns["dma_gather"]
    fn._relaxed = True
    bass.BassGpSimd.dma_gather = fn


_patch_dma_gather()


# ---------------------------------------------------------------- host prep
def _prepare(x, edge_index):
    N = x.shape[0]
    assert N % C == 0
    NPC = N // C
    NPAD = ((NPC + 1 + P - 1) // P) * P      # >=1 fake (zero) row, 128-aligned
    n_chunks = NPAD // P
    TBL = C * NPAD
    n_win = (TBL // 2 + WIN_PAIRS - 1) // WIN_PAIRS
    n_cat = n_win * 2
    n_cells = NPAD // CELL

    src = np.asarray(edge_index[0], dtype=np.int64)
    dst = np.asarray(edge_index[1], dtype=np.int64)
    loops = np.arange(N, dtype=np.int64)
    src2 = np.concatenate([src, loops])
    dst2 = np.concatenate([dst, loops])

    deg = np.bincount(dst2, minlength=N)
    dinv = np.zeros(N, dtype=np.float64)
    nz = deg > 0
    dinv[nz] = 1.0 / np.sqrt(deg[nz].astype(np.float64))
    dinv_pad = np.zeros((C, NPAD), dtype=np.float32)
    for c in range(C):
        dinv_pad[c, :NPC] = dinv[c * NPC:(c + 1) * NPC].astype(np.float32)

    gpos_src = (src2 // NPC) * NPAD + (src2 % NPC)
    gpos_dst = (dst2 // NPC) * NPAD + (dst2 % NPC)

    # a fake (all-zero) row for each (window, parity), for padding slots
    zpad = np.full((n_win, 2), -1, dtype=np.int64)
    for c in range(C):
        for f in range(c * NPAD + NPC, (c + 1) * NPAD):
            zpad[(f // 2) // WIN_PAIRS, f % 2] = f
    assert (zpad >= 0).all(), "every (window,parity) needs a zero row"

    cat_src = ((gpos_src // 2) // WIN_PAIRS) * 2 + (gpos_src % 2)

    # per (core, cat, cell) edge counts -> unified column schedule
    e_core = gpos_dst // NPAD
    e_cell = (gpos_dst % NPAD) // CELL
    flat = (e_core * n_cat + cat_src) * n_cells + e_cell
    counts = np.bincount(flat, minlength=C * n_cat * n_cells).reshape(
        C, n_cat, n_cells)
    ucols = ((counts + P - 1) // P).max(axis=0)          # [n_cat, n_cells]

    # column layout: cat-major, cell order, ucols columns per cell
    cell_col0 = np.zeros((n_cat, n_cells), dtype=np.int64)
    cat_col0 = np.zeros(n_cat + 1, dtype=np.int64)
    pos = 0
    for k in range(n_cat):
        cat_col0[k] = pos
        for ci in range(n_cells):
            cell_col0[k, ci] = pos
            pos += int(ucols[k, ci])
    cat_col0[n_cat] = pos
    TOTCOLS = int(pos)

    # fill slot arrays: [C, TOTCOLS*128] (slot i of a column c at flat
    # position c*128 + (i within column)); dma_gather maps flat slot s to
    # (partition s%128, column s//128), so per-column slots sit on partitions.
    idx_all = np.empty((C, TOTCOLS * P), np.int64)
    dstl_all = np.full((C, TOTCOLS * P), float(CELL), np.float32)
    for k in range(n_cat):
        w = k // 2
        zrow = zpad[w, k % 2]
        idx_all[:, cat_col0[k] * P:cat_col0[k + 1] * P] = zrow

    order = np.lexsort((gpos_dst, cat_src, e_core))
    sp_s = gpos_src[order]
    dp_s = gpos_dst[order]
    core_s = e_core[order]
    cat_s = cat_src[order]
    cell_s = (dp_s % NPAD) // CELL
    # position within (core, cat, cell) run
    key = (core_s * n_cat + cat_s) * n_cells + cell_s
    first = np.r_[True, key[1:] != key[:-1]]
    run_start = np.flatnonzero(first)
    run_id = np.cumsum(first) - 1
    within = np.arange(len(key)) - run_start[run_id]

    slotpos = cell_col0[cat_s, cell_s] * P + within
    idx_all[core_s, slotpos] = sp_s
    dstl_all[core_s, slotpos] = (dp_s % NPAD) % CELL

    # window-local pair-row indices (int16)
    win_of = (idx_all // 2) // WIN_PAIRS
    loc = idx_all // 2 - win_of * WIN_PAIRS
    assert (loc >= 0).all() and (loc < WIN_PAIRS).all()
    idx16 = loc.astype(np.int16)

    # per-column static metadata (same for every core)
    col_meta = []          # (cat, block, half, start, stop, add_after)
    first_contrib = np.ones((n_cells,), dtype=bool)
    cell_first = np.zeros((n_cat, n_cells), dtype=bool)
    for k in range(n_cat):
        for ci in range(n_cells):
            nc_ = int(ucols[k, ci])
            if nc_ == 0:
                continue
            if first_contrib[ci]:
                cell_first[k, ci] = True
                first_contrib[ci] = False
            for j in range(nc_):
                col_meta.append((k, ci // 2, ci % 2, j == 0, j == nc_ - 1,
                                 j == nc_ - 1))

    return dict(
        N=N, NPC=NPC, NPAD=NPAD, n_chunks=n_chunks, TBL=TBL,
        n_win=n_win, n_cat=n_cat, n_cells=n_cells,
        dinv_pad=dinv_pad, idx16=idx16, dstl=dstl_all,
        ucols=ucols, cat_col0=cat_col0, cell_col0=cell_col0,
        col_meta=col_meta, cell_first=cell_first, TOTCOLS=TOTCOLS,
    )


# ---------------------------------------------------------------- bass build
def _build(plan):
    NPAD = plan["NPAD"]
    n_chunks = plan["n_chunks"]
    n_cat = plan["n_cat"]
    n_cells = plan["n_cells"]
    TBL = plan["TBL"]
    TOTCOLS = plan["TOTCOLS"]
    cat_col0 = plan["cat_col0"]
    ucols = plan["ucols"]
    cell_first = plan["cell_first"]

    nc = bacc.Bacc("TRN2", target_bir_lowering=False, debug=False,
                   num_devices=C)

    xT_t = nc.dram_tensor("xT", [F_IN, NPAD], F32, kind="ExternalInput")
    idx_t = nc.dram_tensor("idx", [P, TOTCOLS * 8], I16, kind="ExternalInput")
    dstl_t = nc.dram_tensor("dstl", [P, TOTCOLS], F32, kind="ExternalInput")
    dinv_t = nc.dram_tensor("dinv", [P, n_chunks], F32, kind="ExternalInput")
    iota_t = nc.dram_tensor("iota", [P, CELL], F32, kind="ExternalInput")
    W1_t = nc.dram_tensor("W1", [F_IN, H], F32, kind="ExternalInput")
    W2_t = nc.dram_tensor("W2", [H, H], F32, kind="ExternalInput")
    W3_t = nc.dram_tensor("W3", [H, H], F32, kind="ExternalInput")
    Wl_t = nc.dram_tensor("Wl", [H, F_IN], F32, kind="ExternalInput")
    brep_t = nc.dram_tensor("brep", [P, 3 * H], F32, kind="ExternalInput")
    blin_t = nc.dram_tensor("blin", [P, F_IN], F32, kind="ExternalInput")
    out_t = nc.dram_tensor("out", [NPAD, F_IN], F32, kind="ExternalOutput")

    tables = [nc.dram_tensor(f"table{l}", [TBL, H], F32) for l in range(3)]
    agins = [nc.dram_tensor(f"agin{l}", [NPAD, H], F32) for l in range(3)]
    xbufs = [nc.dram_tensor(f"xb{i}", [NPAD, H], F32) for i in range(2)]

    Sig = mybir.ActivationFunctionType.Sigmoid
    ISEQ = mybir.AluOpType.is_equal

    with tile.TileContext(nc) as tc:
        with (
            tc.tile_pool(name="cst", bufs=1) as cst,
            tc.tile_pool(name="sb", bufs=3) as sb,
            tc.tile_pool(name="gth", bufs=2) as gp,
            tc.tile_pool(name="ps", bufs=4, space="PSUM") as ps,
            tc.tile_pool(name="ppool", bufs=3, space="PSUM") as pp,
        ):
            ident = cst.tile([P, P], F32)
            make_identity(nc, ident[:])
            w1 = cst.tile([F_IN, H], F32)
            nc.sync.dma_start(out=w1[:], in_=W1_t.ap())
            w2 = cst.tile([H, H], F32)
            nc.sync.dma_start(out=w2[:], in_=W2_t.ap())
            w3 = cst.tile([H, H], F32)
            nc.sync.dma_start(out=w3[:], in_=W3_t.ap())
            wl = cst.tile([H, F_IN], F32)
            nc.sync.dma_start(out=wl[:], in_=Wl_t.ap())
            brep = cst.tile([P, 3 * H], F32)
            nc.sync.dma_start(out=brep[:], in_=brep_t.ap())
            blin = cst.tile([P, F_IN], F32)
            nc.sync.dma_start(out=blin[:], in_=blin_t.ap())
            dinv_sb = cst.tile([P, n_chunks], F32)
            nc.sync.dma_start(out=dinv_sb[:], in_=dinv_t.ap())
            iota = cst.tile([P, CELL], F32)
            nc.sync.dma_start(out=iota[:], in_=iota_t.ap())
            acc = cst.tile([P, n_chunks * H], F32)
            nc.vector.memset(acc[:], 0.0)

            def mm_phase(layer):
                w = (w1, w2, w3)[layer]
                for i in range(n_chunks):
                    if layer == 0:
                        lhsT = sb.tile([F_IN, P], F32, tag="xt")
                        nc.sync.dma_start(
                            out=lhsT[:], in_=xT_t.ap()[:, i * P:(i + 1) * P])
                    else:
                        xc = sb.tile([P, H], F32, tag="xc")
                        nc.sync.dma_start(
                            out=xc[:],
                            in_=xbufs[(layer + 1) % 2].ap()[i * P:(i + 1) * P, :])
                        tp = ps.tile([H, P], F32, tag="u")
                        nc.tensor.transpose(
                            out=tp[:], in_=xc[:], identity=ident[:])
                        lhsT = sb.tile([H, P], F32, tag="xtT")
                        nc.vector.tensor_copy(out=lhsT[:], in_=tp[:])
                    pt = ps.tile([P, H], F32, tag="u")
                    nc.tensor.matmul(
                        out=pt[:], lhsT=lhsT[:], rhs=w[:], start=True, stop=True)
                    hh = sb.tile([P, H], F32, tag="hh")
                    nc.vector.tensor_scalar_mul(
                        hh[:], pt[:], dinv_sb[:, i:i + 1])
                    nc.sync.dma_start(
                        out=agins[layer].ap()[i * P:(i + 1) * P, :], in_=hh[:])

            def prop_phase(layer, dst_xb):
                pv = tables[layer].ap().rearrange("(q two) f -> q (two f)",
                                                  two=2)
                n_pairs = TBL // 2
                for k in range(n_cat):
                    w, par = k // 2, k % 2
                    rows = min(WIN_PAIRS, n_pairs - w * WIN_PAIRS)
                    in_ap = pv[w * WIN_PAIRS:w * WIN_PAIRS + rows,
                               par * H:(par + 1) * H]
                    c0, c1 = int(cat_col0[k]), int(cat_col0[k + 1])
                    pt = None
                    pt_blk = -1
                    for call0 in range(c0, c1, TCOLS):
                        ncols = min(TCOLS, c1 - call0)
                        it = sb.tile([P, TCOLS * 8], I16, tag="idx")
                        nc.sync.dma_start(
                            out=it[:, :ncols * 8],
                            in_=idx_t.ap()[:, call0 * 8:(call0 + ncols) * 8])
                        dl = sb.tile([P, TCOLS], F32, tag="dl")
                        nc.sync.dma_start(
                            out=dl[:, :ncols],
                            in_=dstl_t.ap()[:, call0:call0 + ncols])
                        g = gp.tile([P, TCOLS * H], F32, tag="g")
                        nc.gpsimd.dma_gather(
                            out_ap=g[:, :ncols * H].rearrange(
                                "p (c e) -> p c e", e=H),
                            in_ap=in_ap,
                            idxs_ap=it[:, :ncols * 8],
                            num_idxs=ncols * P,
                            num_idxs_reg=ncols * P,
                            elem_size=H,
                            elem_step=2 * H,
                            single_packet=False,
                        )
                        for j in range(ncols):
                            col = call0 + j
                            (ck, blk, half, st, sp_, addaf) = \
                                plan_col_meta[col]
                            assert ck == k
                            if pt is None or blk != pt_blk:
                                pt = pp.tile([P, H], F32, tag="pp")
                                pt_blk = blk
                            S = sb.tile([P, CELL], F32, tag="S")
                            nc.vector.tensor_scalar(
                                S[:], iota[:], dl[:, j:j + 1], None, op0=ISEQ)
                            nc.tensor.matmul(
                                out=pt[half * CELL:(half + 1) * CELL, :],
                                lhsT=S[:],
                                rhs=g[:, j * H:(j + 1) * H],
                                start=st, stop=sp_)
                            if addaf:
                                ci = blk * 2 + half
                                a_sl = acc[half * CELL:(half + 1) * CELL,
                                           blk * H:(blk + 1) * H]
                                p_sl = pt[half * CELL:(half + 1) * CELL, :]
                                if cell_first[k, ci]:
                                    nc.vector.tensor_copy(out=a_sl, in_=p_sl)
                                else:
                                    nc.vector.tensor_add(a_sl, a_sl, p_sl)
                # epilogue: out_d = act(dinv*acc + b) per 128-chunk
                for i in range(n_chunks):
                    t2 = sb.tile([P, H], F32, tag="t2")
                    nc.vector.tensor_scalar_mul(
                        t2[:], acc[:, i * H:(i + 1) * H], dinv_sb[:, i:i + 1])
                    nc.vector.tensor_add(
                        t2[:], t2[:], brep[:, layer * H:(layer + 1) * H])
                    if layer < 2:
                        xn = sb.tile([P, H], F32, tag="xn")
                        nc.scalar.activation(xn[:], t2[:], Sig)
                    else:
                        xn = t2
                    nc.sync.dma_start(
                        out=dst_xb.ap()[i * P:(i + 1) * P, :], in_=xn[:])

            plan_col_meta = {}
            pos = 0
            for k in range(n_cat):
                for ci in range(n_cells):
                    nc_ = int(ucols[k, ci])
                    for j in range(nc_):
                        plan_col_meta[pos] = (k, ci // 2, ci % 2, j == 0,
                                              j == nc_ - 1, j == nc_ - 1)
                        pos += 1
            assert pos == TOTCOLS

            rg = [list(range(C))]
            no_cc = os.environ.get("GCN_NO_COLLECTIVE", "0") == "1"
            n_layers = int(os.environ.get("GCN_LAYERS", "3"))
            for layer in range(n_layers):
                mm_phase(layer)
                if no_cc:
                    # debug: copy own chunk into slot 0 of the table instead
                    tcp = sb.tile([P, H], F32, tag="tcp")
                    for i in range(n_chunks):
                        nc.sync.dma_start(
                            out=tcp[:],
                            in_=agins[layer].ap()[i * P:(i + 1) * P, :])
                        nc.sync.dma_start(
                            out=tables[layer].ap()[i * P:(i + 1) * P, :],
                            in_=tcp[:])
                else:
                    nc.gpsimd.collective_compute(
                        "AllGather",
                        mybir.AluOpType.bypass,
                        replica_groups=rg,
                        ins=[agins[layer].ap().opt()],
                        outs=[tables[layer].ap().opt()],
                    )
                if os.environ.get("GCN_SKIP_PROP", "0") != "1":
                    prop_phase(layer, xbufs[layer % 2])

            fxb = xbufs[0] if n_layers % 2 == 1 else xbufs[1]
            skip_final = os.environ.get("GCN_SKIP_FINAL", "0") == "1"
            final_chunks = 0 if skip_final else n_chunks
            if skip_final:
                for i in range(n_chunks):
                    xc = sb.tile([P, F_IN], F32, tag="fxc2")
                    nc.sync.dma_start(
                        out=xc[:, :H], in_=fxb.ap()[i * P:(i + 1) * P, :])
                    nc.sync.dma_start(
                        out=out_t.ap()[i * P:(i + 1) * P, :], in_=xc[:])
            for i in range(final_chunks):
                xc = sb.tile([P, H], F32, tag="fxc")
                nc.sync.dma_start(
                    out=xc[:], in_=fxb.ap()[i * P:(i + 1) * P, :])
                tp = ps.tile([H, P], F32, tag="u")
                nc.tensor.transpose(out=tp[:], in_=xc[:], identity=ident[:])
                xtT = sb.tile([H, P], F32, tag="fxtT")
                nc.vector.tensor_copy(out=xtT[:], in_=tp[:])
                pf = ps.tile([P, F_IN], F32, tag="u")
                nc.tensor.matmul(
                    out=pf[:], lhsT=xtT[:], rhs=wl[:], start=True, stop=True)
                of = sb.tile([P, F_IN], F32, tag="of")
                nc.vector.tensor_add(of[:], pf[:], blin[:])
                o2 = sb.tile([P, F_IN], F32, tag="o2")
                nc.scalar.activation(o2[:], of[:], Sig)
                nc.sync.dma_start(
                    out=out_t.ap()[i * P:(i + 1) * P, :], in_=o2[:])

    nc.compile()
    return nc


# ---------------------------------------------------------------- entry
_CACHE = {}


def kernel(x, edge_index, W1, b1, W2, b2, W3, b3, Wlin, blin):
    x = np.asarray(x, dtype=np.float32)
    edge_index = np.asarray(edge_index)
    W1 = np.asarray(W1, dtype=np.float32)
    b1 = np.asarray(b1, dtype=np.float32)
    W2 = np.asarray(W2, dtype=np.float32)
    b2 = np.asarray(b2, dtype=np.float32)
    W3 = np.asarray(W3, dtype=np.float32)
    b3 = np.asarray(b3, dtype=np.float32)
    Wlin = np.asarray(Wlin, dtype=np.float32)
    blin = np.asarray(blin, dtype=np.float32)

    plan = _prepare(x, edge_index)
    N, NPC, NPAD = plan["N"], plan["NPC"], plan["NPAD"]

    key = (N, edge_index.shape[1], plan["TOTCOLS"])
    if key not in _CACHE:
        _CACHE[key] = _build(plan)
    nc = _CACHE[key]

    brep = np.concatenate([
        np.tile(b1[None, :], (P, 1)),
        np.tile(b2[None, :], (P, 1)),
        np.tile(b3[None, :], (P, 1)),
    ], axis=1).astype(np.float32)
    blin_rep = np.tile(blin[None, :], (P, 1)).astype(np.float32)
    iota = np.tile(np.arange(CELL, dtype=np.float32)[None, :], (P, 1))

    in_maps = []
    for c in range(C):
        xT = np.zeros((F_IN, NPAD), dtype=np.float32)
        xT[:, :NPC] = x[c * NPC:(c + 1) * NPC].T
        # slot s of column j at flat position j*128+s%... -> [s%128, j]
        idxc = plan["idx16"][c].reshape(plan["TOTCOLS"] * 8, 16).T  # [16, .]
        idxc = np.tile(idxc, (8, 1))
        dstlc = plan["dstl"][c].reshape(plan["TOTCOLS"], P).T
        in_maps.append({
            "xT": xT,
            "idx": np.ascontiguousarray(idxc),
            "dstl": np.ascontiguousarray(dstlc),
            "dinv": np.ascontiguousarray(
                plan["dinv_pad"][c].reshape(plan["n_chunks"], P).T),
            "iota": iota,
            "W1": W1, "W2": W2, "W3": W3, "Wl": Wlin,
            "brep": brep, "blin": blin_rep,
        })

    mode = os.environ.get("GCN_BASS_MODE", "hw")
    if mode == "sim":
        from concourse.bass_interp import MultiCoreSim
        sim = MultiCoreSim(nc, C)
        for c in range(C):
            for name, arr in in_maps[c].items():
                sim.cores[c].tensor(name)[:] = arr
        sim.simulate(check_with_hw=False)
        outs = [np.array(sim.cores[c].mem_tensor("out")) for c in range(C)]
    else:
        res = bass_utils.run_bass_kernel_spmd(nc, in_maps, list(range(C)))
        outs = [res.results[c]["out"] for c in range(C)]

    full = np.empty((N, F_IN), dtype=np.float32)
    for c in range(C):
        full[c * NPC:(c + 1) * NPC] = outs[c][:NPC]
    return full



# revision 5
# speedup vs baseline: 3.4483x; 3.4483x over previous
"""3-layer GCN (PyG GCNConv-style) on 8 Trainium2 NeuronCores via Bass/Tile. v2.

reference:  h1 = sigmoid(gcn(x,  W1,b1));  h2 = sigmoid(gcn(h1, W2,b2))
            h3 = gcn(h2, W3,b3);           out = sigmoid(h3 @ Wlin + blin)
with gcn(x,W,b) = D^-1/2 (A+I) D^-1/2 (x W) + b.

Per-edge factorization (norm_e = dinv[src]*dinv[dst]):
    table = dinv * (x W)                        (bf16, per-node row scale)
    sum_d = table[d] + sum_{e: dst(e)=d} table[src(e)]   (self loop folded in)
    out_d = dinv[d]*sum_d + b                   (then sigmoid)

Sharding: nodes block-partitioned across 8 cores. Each core computes its table
chunk (bf16), AllGathers (1.6MB contribution), then gathers + reduces its own
in-edges with dma_gather + one PE matmul per 128-edge column against a
selection matrix S[e,d] = (dstl[e] == d).

v2 vs v1:
 - bf16 table/gather: 64B rows, 256B-stride groups of 4 nodes; int16 index
   reaches 32768 groups = 131072 nodes -> 2 windows x 4 parities = 8 cats.
 - CELL=128 (= dst chunk): pad 35% -> 18%.
 - S built in batches of 32 columns with ONE tensor_tensor is_equal
   (broadcast APs), not per-column tensor_scalar.
 - self-loops folded into the epilogue (reads resident hh), not gathered.
 - epilogue fused: tensor_add + scalar_tensor_tensor + ACT sigmoid.
 - dl, h, hh, acc resident in SBUF; dl/idx shared across the 3 layers.
 - gathers round-robined over num_swdge_queues SWDGE queues.
"""

import os
import textwrap
import inspect
import numpy as np
import ml_dtypes

import concourse.bass as bass
import concourse.bacc as bacc
import concourse.mybir as mybir
import concourse.tile as tile
from concourse import bass_utils
from concourse.masks import make_identity

F32 = mybir.dt.float32
BF16 = mybir.dt.bfloat16
I16 = mybir.dt.int16

C = 8          # cores
P = 128        # partitions
F_IN = 128
H = 32
CELL = 128     # dst nodes per selection cell == chunk
WIN = 32768    # 4-node groups reachable by one int16 index
TCOLS = 64     # columns (of 128 edges) per dma_gather call
SB = 32        # columns per batched S-build


def _patch_dma_gather():
    """bass.dma_gather asserts elem_size_bytes % 256 == 0 (a transpose-path
    restriction applied unconditionally). The ucode handles 64B/128B elements
    with a 256B-encoded stride (verified on device); relax to %64."""
    if getattr(bass.BassGpSimd.dma_gather, "_relaxed", False):
        return
    src = textwrap.dedent(inspect.getsource(bass.BassGpSimd.dma_gather))
    assert "elem_size_bytes % 256 == 0" in src
    src = src.replace("elem_size_bytes % 256 == 0", "elem_size_bytes % 64 == 0")
    ns = {}
    exec(compile(src, "<dma_gather_patched>", "exec"), vars(bass).copy(), ns)
    fn = ns["dma_gather"]
    fn._relaxed = True
    bass.BassGpSimd.dma_gather = fn


_patch_dma_gather()


# ---------------------------------------------------------------- host prep
def _prepare(x, edge_index):
    N = x.shape[0]
    assert N % C == 0
    NPC = N // C
    NPAD = ((NPC + 1 + P - 1) // P) * P
    n_chunks = NPAD // P
    TBL = C * NPAD
    n_grp = TBL // 4
    n_win = (n_grp + WIN - 1) // WIN
    n_cat = n_win * 4
    n_cells = NPAD // CELL

    src = np.asarray(edge_index[0], dtype=np.int64)
    dst = np.asarray(edge_index[1], dtype=np.int64)

    # degree includes the self loop (+1 per node)
    deg = np.bincount(dst, minlength=N) + 1
    dinv = (1.0 / np.sqrt(deg.astype(np.float64))).astype(np.float32)
    dinv_pad = np.zeros((C, NPAD), dtype=np.float32)
    for c in range(C):
        dinv_pad[c, :NPC] = dinv[c * NPC:(c + 1) * NPC]

    core = dst // NPC
    gpos = (src // NPC) * NPAD + (src % NPC)
    grp = gpos // 4
    win = grp // WIN
    cat = win * 4 + gpos % 4
    loc16 = grp - win * WIN                 # int16 index value
    dloc = dst % NPC
    cell = dloc // CELL
    dl = (dloc % CELL).astype(np.float32)

    flat = (core * n_cat + cat) * n_cells + cell
    counts = np.bincount(flat, minlength=C * n_cat * n_cells).reshape(
        C, n_cat, n_cells)
    ucols = ((counts + P - 1) // P).max(axis=0)          # [n_cat, n_cells]

    cell_col0 = np.zeros((n_cat, n_cells), dtype=np.int64)
    cat_col0 = np.zeros(n_cat + 1, dtype=np.int64)
    pos = 0
    for k in range(n_cat):
        cat_col0[k] = pos
        for ci in range(n_cells):
            cell_col0[k, ci] = pos
            pos += int(ucols[k, ci])
    cat_col0[n_cat] = pos
    TOTCOLS = int(pos)

    idx_all = np.zeros((C, TOTCOLS * P), np.int16)       # pad -> group 0
    dl_all = np.full((C, TOTCOLS * P), float(CELL), np.float32)

    order = np.lexsort((dst, cat, core))
    core_s = core[order]
    cat_s = cat[order]
    cell_s = cell[order]
    loc_s = loc16[order]
    dl_s = dl[order]
    key = (core_s * n_cat + cat_s) * n_cells + cell_s
    first = np.r_[True, key[1:] != key[:-1]]
    run_start = np.flatnonzero(first)
    run_id = np.cumsum(first) - 1
    within = np.arange(len(key)) - run_start[run_id]
    slotpos = cell_col0[cat_s, cell_s] * P + within
    idx_all[core_s, slotpos] = loc_s.astype(np.int16)
    dl_all[core_s, slotpos] = dl_s

    # per-column metadata (same for every core): (cell, start, stop)
    col_meta = []
    cell_first = np.zeros((n_cat, n_cells), dtype=bool)
    seen = np.zeros((n_cells,), dtype=bool)
    for k in range(n_cat):
        for ci in range(n_cells):
            nc_ = int(ucols[k, ci])
            if nc_ == 0:
                continue
            if not seen[ci]:
                cell_first[k, ci] = True
                seen[ci] = True
            for j in range(nc_):
                col_meta.append((ci, j == 0, j == nc_ - 1))
    assert seen.all(), "every dst chunk needs at least one column"
    assert len(col_meta) == TOTCOLS

    return dict(
        N=N, NPC=NPC, NPAD=NPAD, n_chunks=n_chunks, TBL=TBL,
        n_grp=n_grp, n_win=n_win, n_cat=n_cat, n_cells=n_cells,
        dinv_pad=dinv_pad, idx16=idx_all, dl=dl_all,
        ucols=ucols, cat_col0=cat_col0, col_meta=col_meta,
        cell_first=cell_first, TOTCOLS=TOTCOLS,
    )


# ---------------------------------------------------------------- bass build
def _build(plan, n_queues):
    NPAD = plan["NPAD"]
    n_chunks = plan["n_chunks"]
    n_cat = plan["n_cat"]
    TBL = plan["TBL"]
    n_grp = plan["n_grp"]
    TOTCOLS = plan["TOTCOLS"]
    cat_col0 = plan["cat_col0"]
    col_meta = plan["col_meta"]
    cell_first = plan["cell_first"]

    nc = bacc.Bacc("TRN2", target_bir_lowering=False, debug=False,
                   num_devices=C, num_swdge_queues=n_queues)

    xT_t = nc.dram_tensor("xT", [F_IN, NPAD], F32, kind="ExternalInput")
    idx_t = nc.dram_tensor("idx", [P, TOTCOLS * 8], I16, kind="ExternalInput")
    dl_t = nc.dram_tensor("dl", [P, TOTCOLS], BF16, kind="ExternalInput")
    dinv_t = nc.dram_tensor("dinv", [P, n_chunks], F32, kind="ExternalInput")
    iota_t = nc.dram_tensor("iota", [P, CELL], BF16, kind="ExternalInput")
    W1_t = nc.dram_tensor("W1", [F_IN, H], F32, kind="ExternalInput")
    W2_t = nc.dram_tensor("W2", [H, H], F32, kind="ExternalInput")
    W3_t = nc.dram_tensor("W3", [H, H], F32, kind="ExternalInput")
    Wl_t = nc.dram_tensor("Wl", [H, F_IN], F32, kind="ExternalInput")
    brep_t = nc.dram_tensor("brep", [P, 3 * H], F32, kind="ExternalInput")
    blin_t = nc.dram_tensor("blin", [P, F_IN], F32, kind="ExternalInput")
    out_t = nc.dram_tensor("out", [NPAD, F_IN], F32, kind="ExternalOutput")

    agins = [nc.dram_tensor(f"agin{l}", [NPAD, H], BF16) for l in range(3)]
    tables = [nc.dram_tensor(f"table{l}", [TBL, H], BF16) for l in range(3)]

    Sig = mybir.ActivationFunctionType.Sigmoid
    Cpy = mybir.ActivationFunctionType.Copy
    ISEQ = mybir.AluOpType.is_equal
    MULT = mybir.AluOpType.mult
    ADD = mybir.AluOpType.add

    with tile.TileContext(nc) as tc:
        with (
            tc.tile_pool(name="cst", bufs=1) as cst,
            tc.tile_pool(name="res", bufs=1) as res,
            tc.tile_pool(name="sb", bufs=3) as sb,
            tc.tile_pool(name="gp", bufs=6) as gp,
            tc.tile_pool(name="sp", bufs=4) as sp,
            tc.tile_pool(name="ip", bufs=6) as ip,
            tc.tile_pool(name="ps", bufs=4, space="PSUM") as ps,
            tc.tile_pool(name="pp", bufs=4, space="PSUM") as pp,
        ):
            ident = cst.tile([P, P], F32)
            make_identity(nc, ident[:])
            w1 = cst.tile([F_IN, H], F32)
            nc.sync.dma_start(out=w1[:], in_=W1_t.ap())
            w2 = cst.tile([H, H], F32)
            nc.sync.dma_start(out=w2[:], in_=W2_t.ap())
            w3 = cst.tile([H, H], F32)
            nc.sync.dma_start(out=w3[:], in_=W3_t.ap())
            wl = cst.tile([H, F_IN], F32)
            nc.sync.dma_start(out=wl[:], in_=Wl_t.ap())
            brep = cst.tile([P, 3 * H], F32)
            nc.sync.dma_start(out=brep[:], in_=brep_t.ap())
            blin = cst.tile([P, F_IN], F32)
            nc.sync.dma_start(out=blin[:], in_=blin_t.ap())
            dinv_sb = cst.tile([P, n_chunks], F32)
            nc.sync.dma_start(out=dinv_sb[:], in_=dinv_t.ap())
            iota = cst.tile([P, CELL], BF16)
            nc.sync.dma_start(out=iota[:], in_=iota_t.ap())
            dlr = res.tile([P, TOTCOLS], BF16)
            nc.sync.dma_start(out=dlr[:], in_=dl_t.ap())

            h_a = res.tile([P, n_chunks * H], F32)
            h_b = res.tile([P, n_chunks * H], F32)
            hh = res.tile([P, n_chunks * H], F32)
            acc = res.tile([P, n_chunks * H], F32)

            _ia = iota[:]

            def mm_phase(layer, h_src):
                w = (w1, w2, w3)[layer]
                for ci in range(n_chunks):
                    if layer == 0:
                        lhsT = sb.tile([F_IN, P], F32, tag="lx")
                        nc.sync.dma_start(
                            out=lhsT[:], in_=xT_t.ap()[:, ci * P:(ci + 1) * P])
                    else:
                        tp = ps.tile([H, P], F32, tag="u")
                        nc.tensor.transpose(
                            out=tp[:], in_=h_src[:, ci * H:(ci + 1) * H],
                            identity=ident[:])
                        lhsT = sb.tile([H, P], F32, tag="lh")
                        nc.vector.tensor_copy(out=lhsT[:], in_=tp[:])
                    pt = ps.tile([P, H], F32, tag="u")
                    nc.tensor.matmul(
                        out=pt[:], lhsT=lhsT[:], rhs=w[:], start=True,
                        stop=True)
                    # hh chunk = dinv * (x W)   (table row values, f32)
                    nc.scalar.activation(
                        hh[:, ci * H:(ci + 1) * H], pt[:], Cpy,
                        scale=dinv_sb[:, ci:ci + 1])
                    hb = sb.tile([P, H], BF16, tag="hb")
                    nc.vector.tensor_copy(
                        out=hb[:], in_=hh[:, ci * H:(ci + 1) * H])
                    nc.sync.dma_start(
                        out=agins[layer].ap()[ci * P:(ci + 1) * P, :],
                        in_=hb[:])

            rg = [list(range(C))]

            def prop_phase(layer, h_dst):
                pv = tables[layer].ap().rearrange(
                    "(q four) f -> q (four f)", four=4)
                qrr = [0]
                pt_state = {"pt": None, "cell": -1}
                for k in range(n_cat):
                    win, par = k // 4, k % 4
                    rows = min(WIN, n_grp - win * WIN)
                    in_ap = pv[win * WIN:win * WIN + rows,
                               par * H:(par + 1) * H]
                    c0, c1 = int(cat_col0[k]), int(cat_col0[k + 1])
                    for call0 in range(c0, c1, TCOLS):
                        ncols = min(TCOLS, c1 - call0)
                        it = ip.tile([P, TCOLS * 8], I16, tag="it")
                        nc.sync.dma_start(
                            out=it[:, :ncols * 8],
                            in_=idx_t.ap()[:, call0 * 8:(call0 + ncols) * 8])
                        g = gp.tile([P, TCOLS * H], BF16, tag="g")
                        nc.gpsimd.dma_gather(
                            out_ap=g[:, :ncols * H].rearrange(
                                "p (c e) -> p c e", e=H),
                            in_ap=in_ap,
                            idxs_ap=it[:, :ncols * 8],
                            num_idxs=ncols * P,
                            num_idxs_reg=ncols * P,
                            elem_size=H,
                            elem_step=4 * H,
                            single_packet=False,
                            queue_num=qrr[0],
                        )
                        qrr[0] = (qrr[0] + 1) % n_queues
                        for half0 in range(0, ncols, SB):
                            hc = min(SB, ncols - half0)
                            Sh = sp.tile([P, SB * CELL], BF16, tag="S")
                            in0 = dlr[:, call0 + half0:call0 + half0 + hc] \
                                .broadcast_to([P, hc, CELL])
                            in1 = bass.AP(
                                _ia.tensor, _ia.offset,
                                [list(_ia.ap[0]), [0, hc], list(_ia.ap[1])])
                            nc.vector.tensor_tensor(
                                out=Sh[:, :hc * CELL].rearrange(
                                    "p (c e) -> p c e", e=CELL),
                                in0=in0, in1=in1, op=ISEQ)
                            for jj in range(hc):
                                col = call0 + half0 + jj
                                ci, st, sp_ = col_meta[col]
                                if st:
                                    pt_state["pt"] = pp.tile(
                                        [P, H], F32, tag="pp", name="ppt")
                                    pt_state["cell"] = ci
                                pt = pt_state["pt"]
                                assert pt_state["cell"] == ci
                                nc.tensor.matmul(
                                    out=pt[:],
                                    lhsT=Sh[:, jj * CELL:(jj + 1) * CELL],
                                    rhs=g[:, (half0 + jj) * H:
                                          (half0 + jj + 1) * H],
                                    start=st, stop=sp_)
                                if sp_:
                                    a_sl = acc[:, ci * H:(ci + 1) * H]
                                    if cell_first[k, ci]:
                                        nc.vector.tensor_copy(
                                            out=a_sl, in_=pt[:])
                                    else:
                                        nc.vector.tensor_add(
                                            a_sl, a_sl, pt[:])
                # epilogue: out_d = act(dinv*(acc + hh) + b) per chunk
                for ci in range(n_chunks):
                    v = sb.tile([P, H], F32, tag="v")
                    nc.vector.tensor_add(
                        v[:], acc[:, ci * H:(ci + 1) * H],
                        hh[:, ci * H:(ci + 1) * H])
                    t = sb.tile([P, H], F32, tag="t")
                    nc.vector.scalar_tensor_tensor(
                        out=t[:], in0=v[:], scalar=dinv_sb[:, ci:ci + 1],
                        in1=brep[:, layer * H:(layer + 1) * H],
                        op0=MULT, op1=ADD)
                    hsl = h_dst[:, ci * H:(ci + 1) * H]
                    if layer < 2:
                        nc.scalar.activation(hsl, t[:], Sig)
                    else:
                        nc.vector.tensor_copy(out=hsl, in_=t[:])

            hs = [h_a, h_b, h_a]
            for layer in range(3):
                mm_phase(layer, hs[layer - 1] if layer else None)
                nc.gpsimd.collective_compute(
                    "AllGather",
                    mybir.AluOpType.bypass,
                    replica_groups=rg,
                    ins=[agins[layer].ap().opt()],
                    outs=[tables[layer].ap().opt()],
                )
                prop_phase(layer, hs[layer])

            # final: out = sigmoid(h3 @ Wlin + blin)
            for ci in range(n_chunks):
                tp = ps.tile([H, P], F32, tag="u")
                nc.tensor.transpose(
                    out=tp[:], in_=h_a[:, ci * H:(ci + 1) * H],
                    identity=ident[:])
                lhsT = sb.tile([H, P], F32, tag="fl")
                nc.vector.tensor_copy(out=lhsT[:], in_=tp[:])
                pf = ps.tile([P, F_IN], F32, tag="u")
                nc.tensor.matmul(
                    out=pf[:], lhsT=lhsT[:], rhs=wl[:], start=True, stop=True)
                of = sb.tile([P, F_IN], F32, tag="of")
                nc.vector.tensor_add(of[:], pf[:], blin[:])
                o2 = sb.tile([P, F_IN], F32, tag="o2")
                nc.scalar.activation(o2[:], of[:], Sig)
                nc.sync.dma_start(
                    out=out_t.ap()[ci * P:(ci + 1) * P, :], in_=o2[:])

    nc.compile()
    return nc


# ---------------------------------------------------------------- entry
_CACHE = {}


def kernel(x, edge_index, W1, b1, W2, b2, W3, b3, Wlin, blin):
    x = np.asarray(x, dtype=np.float32)
    edge_index = np.asarray(edge_index)
    W1 = np.asarray(W1, dtype=np.float32)
    b1 = np.asarray(b1, dtype=np.float32)
    W2 = np.asarray(W2, dtype=np.float32)
    b2 = np.asarray(b2, dtype=np.float32)
    W3 = np.asarray(W3, dtype=np.float32)
    b3 = np.asarray(b3, dtype=np.float32)
    Wlin = np.asarray(Wlin, dtype=np.float32)
    blin = np.asarray(blin, dtype=np.float32)

    n_queues = int(os.environ.get("GCN_QUEUES", "4"))
    plan = _prepare(x, edge_index)
    N, NPC, NPAD = plan["N"], plan["NPC"], plan["NPAD"]

    key = (N, edge_index.shape[1], plan["TOTCOLS"], n_queues)
    if key not in _CACHE:
        _CACHE[key] = _build(plan, n_queues)
    nc = _CACHE[key]

    brep = np.concatenate([
        np.tile(b1[None, :], (P, 1)),
        np.tile(b2[None, :], (P, 1)),
        np.tile(b3[None, :], (P, 1)),
    ], axis=1).astype(np.float32)
    blin_rep = np.tile(blin[None, :], (P, 1)).astype(np.float32)
    iota = np.tile(np.arange(CELL, dtype=np.float32)[None, :], (P, 1)) \
        .astype(ml_dtypes.bfloat16)

    TOT = plan["TOTCOLS"]
    in_maps = []
    for c in range(C):
        xT = np.zeros((F_IN, NPAD), dtype=np.float32)
        xT[:, :NPC] = x[c * NPC:(c + 1) * NPC].T
        idxc = plan["idx16"][c].reshape(TOT * 8, 16).T      # [16, TOT*8]
        idxc = np.tile(idxc, (8, 1))
        dlc = plan["dl"][c].reshape(TOT, P).T.astype(ml_dtypes.bfloat16)
        in_maps.append({
            "xT": xT,
            "idx": np.ascontiguousarray(idxc),
            "dl": np.ascontiguousarray(dlc),
            "dinv": np.ascontiguousarray(
                plan["dinv_pad"][c].reshape(plan["n_chunks"], P).T),
            "iota": iota,
            "W1": W1, "W2": W2, "W3": W3, "Wl": Wlin,
            "brep": brep, "blin": blin_rep,
        })

    mode = os.environ.get("GCN_BASS_MODE", "hw")
    if mode == "sim":
        from concourse.bass_interp import MultiCoreSim
        sim = MultiCoreSim(nc, C)
        for c in range(C):
            for name, arr in in_maps[c].items():
                sim.cores[c].tensor(name)[:] = arr
        sim.simulate(check_with_hw=False)
        outs = [np.array(sim.cores[c].mem_tensor("out")) for c in range(C)]
    else:
        res = bass_utils.run_bass_kernel_spmd(nc, in_maps, list(range(C)))
        outs = [res.results[c]["out"] for c in range(C)]

    full = np.empty((N, F_IN), dtype=np.float32)
    for c in range(C):
        full[c * NPC:(c + 1) * NPC] = outs[c][:NPC]
    return full
